# revision 14
# baseline (speedup 1.0000x reference)
"""Trainium2 Bass kernel v6: fused MHA block, sequence-parallel, all
projections sharded, K^T/V AllGathered, fp8 DoubleRow PV.

Sharding: 8 cores = 2 batches x 4 query/token-chunks of 512. Each core:
  - projects Q^T, K^T, V for its OWN 512 tokens only,
  - stores K^T (bf16) and V (fp8e4) shards to DRAM and AllGathers them
    across the 4-core batch group in four pieces ordered by first use:
    K^T[heads 0-7] -> V[heads 0-7] -> K^T[heads 8-15] -> V[heads 8-15],
  - runs 16-head attention for its 512 queries over all 2048 keys,
    pipelined per head-pair dt: QK (bf16, row-tiled concurrent pair)
    -> exp on Scalar (the 133us critical stream, biased by -6.5 so the
    fp8e5 output never saturates; bias cancels in softmax) -> PV in fp8
    DoubleRow (exp in e5m2 for range, V in e4m3 for precision)
    (two 128-key tiles contracted per matmul, ones-column denominator),
  - deferred softmax normalize per dt,
  - output projection, residual, LayerNorm.

Per-core inputs:
  xtq  [1024, 512]  bf16  x[b,chunk].T (Q/K/V projections)
  xq   [512, 1024]  f32   chunk rows of x[b] (residual input)
  wq/wk/wv [1024, 1024] bf16  [c, h*64+d] head-minor
  wo   [1024, 1024] bf16
  bias [16, 128]    f32   additive key bias minus 4.0, natural order
  gamma/beta [1024] bf16
Output: y [512, 1024] f32.
"""

import contextlib

import numpy as np
import ml_dtypes

import concourse.bass as bass
import concourse.tile as tile
from concourse import mybir
from concourse import bass_utils

BF16 = ml_dtypes.bfloat16
N_CORES = 8
B, L, D, H, DH = 2, 2048, 1024, 16, 64
C = 512             # queries/tokens per core (chunk)
CT = D // 128       # contraction tiles over features
JT = L // 128       # key tiles
JP = JT // 2        # key tile pairs (fp8 DoubleRow contraction)
IT = C // 128       # query tiles per core
LN_EPS = 1e-5
EXP_BIAS = -6.5     # exp(s/8 - 6.5): keeps exp in fp8e5 range (max 57344)

F32 = mybir.dt.float32
BF = mybir.dt.bfloat16
F8 = mybir.dt.float8e4
F8E = mybir.dt.float8e5
DR = mybir.MatmulPerfMode.DoubleRow

GROUPS = [[0, 1, 2, 3], [4, 5, 6, 7]]


def _split_waits(nc, maxw=1):
    """This walrus build rejects instructions with more than one sync wait;
    split excess waits into preceding NOPs on the same engine."""
    ctr = 0
    for fn in nc.m.functions:
        for bb in fn.blocks:
            new_insts = []
            for inst in bb.instructions:
                si = inst.sync_info
                if si is not None and len(si.on_wait) > maxw:
                    waits = list(si.on_wait)
                    excess, keep = waits[:-maxw], waits[-maxw:]
                    for i in range(0, len(excess), maxw):
                        ctr += 1
                        new_insts.append(mybir.InstNoOp(
                            name=f"waitsplit_nop_{ctr}",
                            engine=inst.engine,
                            sync_info=mybir.SyncInfo(
                                on_wait=excess[i:i + maxw], on_update=[]),
                            text_hint="waitsplit",
                        ))
                    si.on_wait = keep
                new_insts.append(inst)
            bb.instructions = new_insts
    return ctr


def _emit(nc, tc, hh, masked, sim):
    Exp = mybir.ActivationFunctionType.Exp
    Sqrt = mybir.ActivationFunctionType.Sqrt

    xtq_ap = hh["xtq"].ap().rearrange("(t p) k -> p t k", p=128)  # [128,8,512]
    wq_ap = hh["wq"].ap().rearrange("(t p) d -> p t d", p=128)
    wk_ap = hh["wk"].ap().rearrange("(t p) d -> p t d", p=128)
    wv_ap = hh["wv"].ap().rearrange("(t p) d -> p t d", p=128)
    wo_ap = hh["wo"].ap().rearrange("(t p) d -> p t d", p=128)
    bias_ap = hh["bias"].ap().rearrange("a b -> b a")             # [128,16]
    xq_ap = hh["xq"].ap()
    y_ap = hh["y"].ap()
    # K shard store layout: kb[d*128+p, k] per dt-in-piece d
    kb_aps = [hh["kb01"].ap().rearrange("(t p) k -> p t k", p=128),
              hh["kb23"].ap().rearrange("(t p) k -> p t k", p=128),
              hh["kb47"].ap().rearrange("(t p) k -> p t k", p=128)]
    vb_ap = hh["vb"].ap().rearrange("(t p) k -> p t k", p=128)
    # gathered K^T: [4 cores x n x 128, 512] -> per (core, dt-in-piece)
    kg01_ap = hh["kg01"].ap().rearrange("(c t p) k -> p c t k", p=128, t=2)
    kg23_ap = hh["kg23"].ap().rearrange("(c t p) k -> p c t k", p=128, t=2)
    kg47_ap = hh["kg47"].ap().rearrange("(c t p) k -> p c t k", p=128, t=4)
    # gathered V: [2048, 1024] -> per key tile [128, 16 heads, 64]
    vg_ap = hh["vg"].ap().rearrange("(j p) (h d) -> p j h d", p=128, d=DH)

    def bcast_dram(h1d, parts=128):
        a = h1d.ap()
        return bass.AP(tensor=a.tensor, offset=a.offset,
                       ap=[[0, parts]] + list(a.ap))

    with contextlib.ExitStack() as ctx:
        const = ctx.enter_context(tc.tile_pool(name="const", bufs=1))
        wpool = ctx.enter_context(tc.tile_pool(name="wpool", bufs=6))
        xtp = ctx.enter_context(tc.tile_pool(name="xtp", bufs=1))
        ktp = ctx.enter_context(tc.tile_pool(name="ktp", bufs=1))
        ksg = ctx.enter_context(tc.tile_pool(name="ksg", bufs=1))
        vp = ctx.enter_context(tc.tile_pool(name="vp", bufs=1))
        exp_ = ctx.enter_context(tc.tile_pool(name="exp", bufs=20))
        qtp = ctx.enter_context(tc.tile_pool(name="qtp", bufs=1))
        ptp = ctx.enter_context(tc.tile_pool(name="ptp", bufs=1))
        pv2p = ctx.enter_context(tc.tile_pool(name="pv2p", bufs=2))
        npool = ctx.enter_context(tc.tile_pool(name="npool", bufs=2))
        xqp = ctx.enter_context(tc.tile_pool(name="xqp", bufs=2))
        lnp = ctx.enter_context(tc.tile_pool(name="lnp", bufs=2))
        statp = ctx.enter_context(tc.tile_pool(name="statp", bufs=4))
        psS = ctx.enter_context(tc.tile_pool(name="psS", bufs=2, space="PSUM"))
        psP = ctx.enter_context(tc.tile_pool(name="psP", bufs=1, space="PSUM"))
        psO = ctx.enter_context(tc.tile_pool(name="psO", bufs=1, space="PSUM"))

        # ---- constants / small loads ----
        eps_sb = const.tile([128, 1], F32)
        nc.vector.memset(eps_sb[:], LN_EPS)
        ones64 = const.tile([1, 64], BF)
        nc.vector.memset(ones64[:], 1.0)
        bias_sb = const.tile([128, 16], F32)
        if masked:
            nc.gpsimd.dma_start(out=bias_sb[:], in_=bias_ap)
        nbias_sb = const.tile([128, 1], F32)
        nc.vector.memset(nbias_sb[:], EXP_BIAS)
        gamma_sb = const.tile([128, 1024], BF)
        beta_sb = const.tile([128, 1024], BF)

        # ---- persistent SBUF tensors ----
        xtq_sb = xtp.tile([128, 8, 512], BF)
        qt_all = qtp.tile([128, 8, 512], BF)
        probt = ptp.tile([128, 8, 512], BF)
        kt_all = ktp.tile([128, 8, 2048], BF)        # gathered K^T
        kstage = ksg.tile([128, 8, 512], BF)         # own K^T, staging for AG
        vstage = ksg.tile([128, 4, 1024], F8)        # own V, staging for AG
        # gathered V in fp8 DoubleRow layout [p, jt-pair, ko, head, dh+1]
        v_all = vp.tile([128, JP, 2, H, DH + 1], F8)

        nc.vector.memset(v_all[:, :, :, :, DH:DH + 1], 1.0)

        # initial loads: xtq + wk on the gpsimd queue (it issues DMAs ~10us
        # before the HWDGE rings spin up; K proj gates the whole AG chain)
        nc.gpsimd.dma_start(out=xtq_sb[:], in_=xtq_ap)

        def whalf(w_ap, h, queue):
            t = wpool.tile([128, 8, 512], BF, tag="w", name=f"w{h}")
            queue(out=t[:], in_=w_ap[:, :, h * 512:(h + 1) * 512])
            return t

        wk_h = [whalf(wk_ap, h, nc.gpsimd.dma_start) for h in range(2)]
        wq_h = [whalf(wq_ap, h, nc.scalar.dma_start) for h in range(2)]
        wv_h = [whalf(wv_ap, h, nc.scalar.dma_start) for h in range(2)]

        # ---- K^T projection of own chunk (dt pairs) + shard stores ----
        def ktproj_pair(p2):
            ps = psS.tile([128, 2, 512], F32, tag="ss")
            for half in range(2):
                dt = 2 * p2 + half
                for ct in range(CT):
                    nc.tensor.matmul(
                        ps[:, half, :],
                        wk_h[dt // 4][:, ct, (dt % 4) * 128:(dt % 4 + 1) * 128],
                        xtq_sb[:, ct, :],
                        start=(ct == 0), stop=(ct == CT - 1))
            nc.vector.tensor_copy(kstage[:, 2 * p2:2 * p2 + 2, :], ps[:])
            kb = kb_aps[min(p2, 2)]
            for half in range(2):
                dt = 2 * p2 + half
                nc.sync.dma_start(out=kb[:, dt - (0, 2, 4)[min(p2, 2)], :],
                                  in_=kstage[:, dt, :])

        ktproj_pair(0)
        if not sim:
            nc.gpsimd.collective_compute(
                "AllGather", mybir.AluOpType.bypass, replica_groups=GROUPS,
                ins=[hh["kb01"].ap()], outs=[hh["kg01"].ap()])
        for p2 in range(1, 4):
            ktproj_pair(p2)

        # ---- V projection helper: per (nh, tt-pair) ----
        def vproj_pair(nh, t2):
            ps = psS.tile([128, 2, 512], F32, tag="ss")
            for half in range(2):
                tt = 2 * t2 + half
                for ct in range(CT):
                    nc.tensor.matmul(
                        ps[:, half, :],
                        xtq_sb[:, ct, tt * 128:(tt + 1) * 128],
                        wv_h[nh][:, ct, :],
                        start=(ct == 0), stop=(ct == CT - 1))
            out = vstage[:, 2 * t2:2 * t2 + 2, nh * 512:(nh + 1) * 512]
            nc.vector.tensor_copy(out, ps[:])
            for half in range(2):
                tt = 2 * t2 + half
                nc.sync.dma_start(
                    out=vb_ap[:, tt, nh * 512:(nh + 1) * 512],
                    in_=vstage[:, tt, nh * 512:(nh + 1) * 512])

        # ---- V projection of own chunk (gates the 2nd mesh transfer) ----
        vproj_pair(0, 0)
        vproj_pair(0, 1)
        vproj_pair(1, 0)
        vproj_pair(1, 1)
        if not sim:
            nc.gpsimd.collective_compute(
                "AllGather", mybir.AluOpType.bypass, replica_groups=GROUPS,
                ins=[hh["vb"].ap()], outs=[hh["vg"].ap()])
            nc.gpsimd.collective_compute(
                "AllGather", mybir.AluOpType.bypass, replica_groups=GROUPS,
                ins=[hh["kb23"].ap()], outs=[hh["kg23"].ap()])
            nc.gpsimd.collective_compute(
                "AllGather", mybir.AluOpType.bypass, replica_groups=GROUPS,
                ins=[hh["kb47"].ap()], outs=[hh["kg47"].ap()])

        # ---- Q^T projection (dt pairs 0,1 now; 2,3 as attention fillers) ----
        def qtproj_pair(p2):
            ps = psS.tile([128, 2, 512], F32, tag="ss")
            for half in range(2):
                dt = 2 * p2 + half
                for ct in range(CT):
                    nc.tensor.matmul(
                        ps[:, half, :],
                        wq_h[dt // 4][:, ct, (dt % 4) * 128:(dt % 4 + 1) * 128],
                        xtq_sb[:, ct, :],
                        start=(ct == 0), stop=(ct == CT - 1))
            nc.vector.tensor_copy(qt_all[:, 2 * p2:2 * p2 + 2, :], ps[:])

        qtproj_pair(0)
        qtproj_pair(1)

        # gathered K^T dt0-1 loads (sync queue)
        for cs in range(4):
            nc.sync.dma_start(out=kt_all[:, 0:2, cs * 512:(cs + 1) * 512],
                              in_=kg01_ap[:, cs, :, :])
        # gathered V loads (gpsimd queue, after every mesh trigger)
        for jp in range(JP):
            for ko in range(2):
                nc.gpsimd.dma_start(
                    out=v_all[:, jp, ko, :, 0:DH],
                    in_=vg_ap[:, 2 * jp + ko, :, :])

        # ---- tail prefetch: wo / gamma / beta load during attention ----
        wo_h = [whalf(wo_ap, h, nc.scalar.dma_start) for h in range(2)]
        nc.gpsimd.dma_start(out=gamma_sb[:], in_=bcast_dram(hh["gamma"]))
        nc.gpsimd.dma_start(out=beta_sb[:], in_=bcast_dram(hh["beta"]))

        # ---- attention over head-pairs dt, software-pipelined ----
        prev = {}

        def qk_jt(dt, jt, e):
            ps = psS.tile([128, 2, 512], F32, tag="ss")
            for hb in range(2):
                nc.tensor.matmul(
                    ps[:, hb, :],
                    kt_all[hb * 64:hb * 64 + 64, dt,
                           jt * 128:(jt + 1) * 128],
                    qt_all[hb * 64:hb * 64 + 64, dt, :],
                    start=True, stop=True)
            if masked:
                nc.scalar.activation(
                    e[:, jt % 2, :, :], ps[:], Exp,
                    bias=bias_sb[:, jt:jt + 1], scale=1.0 / 8.0)
            else:
                nc.scalar.activation(
                    e[:, jt % 2, :, :], ps[:], Exp,
                    bias=nbias_sb[:], scale=1.0 / 8.0)

        def pv_jp(dt, jp, pv, es):
            for hb in range(2):
                nc.tensor.matmul(
                    pv[:, hb, :],
                    v_all[:, jp, :, 2 * dt + hb, :],
                    es[jp][:, :, hb, :],
                    start=(jp == 0), stop=(jp == JP - 1),
                    perf_mode=DR)

        def norm_dt(dt, pv):
            # evict numerators + denominator, one fast reciprocal, multiply
            pv2 = pv2p.tile([DH, 2, 512], F32, tag="pv2", name=f"pv2_{dt}")
            den = npool.tile([1, 2, 512], BF, tag="n1", name=f"den{dt}")
            nc.vector.tensor_copy(pv2[:], pv[0:DH, :, :])
            nc.vector.tensor_copy(den[:], pv[DH:DH + 1, :, :])
            bc = psO.tile([128, 512], F32, tag="bc", name=f"bc{dt}")
            nc.tensor.matmul(bc[0:64, :], ones64[:], den[:, 0, :],
                             start=True, stop=True)
            nc.tensor.matmul(bc[64:128, :], ones64[:], den[:, 1, :],
                             start=True, stop=True)
            nc.vector.reciprocal(bc[:], bc[:])
            for hb in range(2):
                nc.vector.tensor_mul(
                    probt[hb * 64:hb * 64 + 64, dt, :], pv2[:, hb, :],
                    bc[hb * 64:hb * 64 + 64, :])

        def fillers(dt):
            if dt == 0:
                for cs in range(4):
                    nc.sync.dma_start(
                        out=kt_all[:, 2:4, cs * 512:(cs + 1) * 512],
                        in_=kg23_ap[:, cs, :, :])
            elif dt == 1:
                for cs in range(4):
                    nc.sync.dma_start(
                        out=kt_all[:, 4:8, cs * 512:(cs + 1) * 512],
                        in_=kg47_ap[:, cs, :, :])
            elif dt == 2:
                qtproj_pair(2)
            elif dt == 3:
                qtproj_pair(3)

        for dt in range(8):
            es = [exp_.tile([128, 2, 2, 512], F8E, name=f"e{dt}_{jp}",
                            tag="e") for jp in range(JP)]
            pv = psP.tile([DH + 1, 2, 512], F32, tag="pp", name=f"pv{dt}")
            for jt in range(JT):
                qk_jt(dt, jt, es[jt // 2])
                if prev and jt % 2 == 1:
                    pv_jp(prev["dt"], jt // 2, prev["pv"], prev["es"])
            if prev:
                norm_dt(prev["dt"], prev["pv"])
            fillers(dt)
            prev = {"dt": dt, "pv": pv, "es": es}
        for jp in range(JP):
            pv_jp(7, jp, prev["pv"], prev["es"])
        norm_dt(7, prev["pv"])

        # ---- output projection + residual + LayerNorm ----
        for it in range(IT):
            xq_t = xqp.tile([128, 1024], F32, tag="xq")
            nc.scalar.dma_start(out=xq_t[:],
                                in_=xq_ap[it * 128:(it + 1) * 128, :])
            ps_r = psS.tile([128, 2, 512], F32, tag="ss")
            for mh in range(2):
                for kt in range(8):
                    nc.tensor.matmul(
                        ps_r[:, mh, :],
                        probt[:, kt, it * 128:(it + 1) * 128],
                        wo_h[mh][:, kt, :],
                        start=(kt == 0), stop=(kt == 7))
            h_sb = lnp.tile([128, 1024], F32, tag="ln")
            nc.vector.tensor_add(h_sb[:], ps_r.rearrange("p a b -> p (a b)"),
                                 xq_t[:])
            stats = statp.tile([128, 2, 6], F32)
            nc.vector.bn_stats(stats[:, 0, :], h_sb[:, 0:512])
            nc.vector.bn_stats(stats[:, 1, :], h_sb[:, 512:1024])
            mv = statp.tile([128, 2], F32)
            nc.vector.bn_aggr(mv[:], stats[:])
            std = statp.tile([128, 1], F32)
            nc.scalar.activation(std[:], mv[:, 1:2], Sqrt,
                                 bias=eps_sb[:], scale=1.0)
            rstd = statp.tile([128, 1], F32)
            nc.vector.reciprocal(rstd[:], std[:])
            t1 = lnp.tile([128, 1024], F32, tag="ln")
            nc.vector.tensor_scalar(
                t1[:], h_sb[:], mv[:, 0:1], rstd[:],
                op0=mybir.AluOpType.subtract, op1=mybir.AluOpType.mult)
            t2 = lnp.tile([128, 1024], F32, tag="ln")
            nc.vector.tensor_mul(t2[:], t1[:], gamma_sb[:])
            out_t = lnp.tile([128, 1024], F32, tag="ln")
            nc.vector.tensor_add(out_t[:], t2[:], beta_sb[:])
            nc.scalar.dma_start(y_ap[it * 128:(it + 1) * 128, :], out_t[:])


def build_module(split=True, masked=False, sim=False):
    nc = bass.Bass("TRN2", target_bir_lowering=False, debug=False,
                   num_devices=N_CORES)
    shard_kind = "ExternalInput" if sim else "Internal"
    hh = {
        "xtq": nc.dram_tensor("xtq", [D, C], BF, kind="ExternalInput"),
        "xq": nc.dram_tensor("xq", [C, D], F32, kind="ExternalInput"),
        "wq": nc.dram_tensor("wq", [D, D], BF, kind="ExternalInput"),
        "wk": nc.dram_tensor("wk", [D, D], BF, kind="ExternalInput"),
        "wv": nc.dram_tensor("wv", [D, D], BF, kind="ExternalInput"),
        "wo": nc.dram_tensor("wo", [D, D], BF, kind="ExternalInput"),
        "bias": nc.dram_tensor("bias", [16, 128], F32, kind="ExternalInput"),
        "gamma": nc.dram_tensor("gamma", [D], BF, kind="ExternalInput"),
        "beta": nc.dram_tensor("beta", [D], BF, kind="ExternalInput"),
        "y": nc.dram_tensor("y", [C, D], F32, kind="ExternalOutput"),
        "kb01": nc.dram_tensor("kb01", [256, C], BF, kind="Internal"),
        "kb23": nc.dram_tensor("kb23", [256, C], BF, kind="Internal"),
        "kb47": nc.dram_tensor("kb47", [C, C], BF, kind="Internal"),
        "vb": nc.dram_tensor("vb", [C, D], F8, kind="Internal"),
        "kg01": nc.dram_tensor("kg01", [1024, C], BF, kind=shard_kind),
        "kg23": nc.dram_tensor("kg23", [1024, C], BF, kind=shard_kind),
        "kg47": nc.dram_tensor("kg47", [L, C], BF, kind=shard_kind),
        "vg": nc.dram_tensor("vg", [L, D], F8, kind=shard_kind),
    }
    with tile.TileContext(nc) as tc:
        _emit(nc, tc, hh, masked, sim)
    if split:
        _split_waits(nc, 1)
    return nc


_CACHE = {}


def get_module(masked=False):
    key = ("nc", masked)
    if key not in _CACHE:
        _CACHE[key] = build_module(masked=masked)
    return _CACHE[key]


def prep_inputs(x, mask, w_q, w_k, w_v, w_o, ln_gamma, ln_beta):
    x = np.asarray(x, dtype=np.float32)
    mask = np.asarray(mask)
    shared = {
        "wq": np.ascontiguousarray(
            np.asarray(w_q, np.float32).transpose(1, 0, 2).reshape(D, D)
        ).astype(BF16),
        "wk": np.ascontiguousarray(
            np.asarray(w_k, np.float32).transpose(1, 0, 2).reshape(D, D)
        ).astype(BF16),
        "wv": np.ascontiguousarray(
            np.asarray(w_v, np.float32).transpose(1, 0, 2).reshape(D, D)
        ).astype(BF16),
        "wo": np.asarray(w_o, np.float32).reshape(D, D).astype(BF16),
        "gamma": np.asarray(ln_gamma, np.float32).astype(BF16),
        "beta": np.asarray(ln_beta, np.float32).astype(BF16),
    }
    in_maps = []
    for c in range(N_CORES):
        b, i = c // 4, c % 4
        q0 = i * C
        m = {
            "xtq": np.ascontiguousarray(x[b, q0:q0 + C, :].T).astype(BF16),
            "xq": np.ascontiguousarray(x[b, q0:q0 + C, :]),
            "bias": (np.where(mask[b], 0.0, -1e9).astype(np.float32)
                     + EXP_BIAS).reshape(16, 128),
        }
        m.update(shared)
        in_maps.append(m)
    masked = not bool(mask.all())
    return in_maps, masked


def sim_extra_inputs(in_map):
    """Host-computed gathered K^T / V shards for single-core CoreSim runs
    (collectives are skipped in the sim build). Batch-0 chunk layout."""
    f8 = ml_dtypes.float8_e4m3
    # reconstruct batch-0 x.T columns from the 4 chunk in_maps
    xt = np.concatenate([m["xtq"].astype(np.float32) for m in in_map[0:4]],
                        axis=1)                       # [1024, 2048]
    wk = in_map[0]["wk"].astype(np.float32)
    wv = in_map[0]["wv"].astype(np.float32)
    ktf = (wk.T @ xt).astype(BF16)                    # [1024 (h d), 2048]
    v = (xt.T @ wv).astype(np.float32)                # [2048, (h d)]
    # kg piece rows: (chunk, dt-in-piece, p)
    ktf = ktf.reshape(8, 128, 4, 512)                 # [dt, p, chunk, k]
    kg = ktf.transpose(2, 0, 1, 3).reshape(4, 8, 128 * 512)
    return {
        "kg01": np.ascontiguousarray(kg[:, 0:2, :]).reshape(1024, C).astype(BF16),
        "kg23": np.ascontiguousarray(kg[:, 2:4, :]).reshape(1024, C).astype(BF16),
        "kg47": np.ascontiguousarray(kg[:, 4:8, :]).reshape(L, C).astype(BF16),
        "vg": np.ascontiguousarray(v).astype(f8),
    }


def assemble(results):
    out = np.empty((B, L, D), dtype=np.float32)
    for c in range(N_CORES):
        b, q0 = c // 4, (c % 4) * C
        out[b, q0:q0 + C, :] = results[c]["y"]
    return out


def run(in_maps, masked=False, **kwargs):
    nc = get_module(masked)
    return bass_utils.run_bass_kernel_spmd(
        nc, in_maps, core_ids=list(range(N_CORES)), **kwargs)


def kernel(x, mask, w_q, w_k, w_v, w_o, ln_gamma, ln_beta):
    in_maps, masked = prep_inputs(x, mask, w_q, w_k, w_v, w_o,
                                  ln_gamma, ln_beta)
    res = run(in_maps, masked)
    return assemble(res.results)
